# revision 41
# baseline (speedup 1.0000x reference)
"""Trainium2 Bass kernel for nn_BktModel (soft-membership BKT HMM forward), v2.

Math restructure (exact; validated in mathcheck.py):
  State m_t[c,s] = la_t[c,s] - a3_t[s]  (shifted log-alpha; a3_{-1}=0, m_{-1}=log_i).
  With w_t = cc_{t+1}*omc_t, Sw_t = sum_c w_t, Rw_t = 1-Sw_t (host-precomputed):
    a2_{t+1}[j] = Wm_t[j] + a3_{t-1}[j]*Sw_t + a3_t[j]*Rw_t,  Wm_t = sum_c w_t*m_{t-1}
    m_t = (m_{t-1} - delta_t) * omc_t,  delta_t = a3_t - a3_{t-1}
  Per-step scalar chain (S=2 collapses logsumexp to softplus):
    evarg_t[t',s] = vpre_t[t',s] + a2_t[t']
    a3_t[s] = evarg_t[0,s] + ln(1 + exp(d_t[s])),  d_t = evarg_t[1,:] - evarg_t[0,:]
  Softplus = ACT Exp then ACT Ln(bias=1) back-to-back in one act-table set, so the
  critical chain per step is [DVE evarg-stt, DVE d] -> [ACT exp, ACT ln1p].
  The lazy split q_{t+1} = Wm_t + a3_{t-1}*Sw_t, z_{t+1} = q_{t+1} + evarg0_t*Rw_t,
  vq2_{t+1} = vpre_{t+1} + z_{t+1} keeps everything else off the chain.
  v3: Wm_t is further decoupled from the fresh state via
    Wm_t = Wm2_t - delta_{t-1}*Sw2_t,  Wm2_t = sum_c w2_t*m_{t-2}, w2_t = w_t*omc_{t-1}
  (w2/Sw2/SwD=Sw-Sw2 host-precomputed) so the per-step DVE cascade
  Wm2 -> q1 -> q -> z -> vq2 starts immediately instead of waiting on
  ln -> delta -> m-update; the m-update drops to the end of the step.
  All output work (a2 = q + a3*RwShift, exp, log-softmax) runs as one batched
  epilogue after the T-loop; a2 inputs (qbuf, a3buf) are written per step.

Sharding: data-parallel over batch, 8 cores x 128 rows (partition dim = batch).
"""

import os
import sys
import threading

import numpy as np

for _p in ("/opt/trn_rl_repo", "/root/.axon_site/_ro/trn_rl_repo"):
    if os.path.isdir(_p) and _p not in sys.path:
        sys.path.append(_p)

B, T, C, K = 1024, 500, 64, 2000
S, O = 2, 2
N_CORES = 8
BL = B // N_CORES          # local batch per core (= 128 partitions)
CH = 25                    # timesteps per streamed chunk
NCH = T // CH
WROW = 128                 # fp16 halves per streamed row: w2(64) omc(64)
VROW = 6                   # fp32 floats per streamed row: vpre(4) dvpre(2)

_cache = {}
_lock = threading.Lock()


def _build_program():
    import concourse.mybir as mybir
    import concourse.tile as tile
    from concourse import bacc

    Act = mybir.ActivationFunctionType

    # Steer Bacc's act-table pass to the set holding BOTH Exp and Ln (the only
    # transcendentals used); avoids any activation-table reloads.
    _orig_tables = bacc.get_activation_tables

    def _tables_combined_exp_ln(arch):
        tabs = _orig_tables(arch)
        out = {}
        for name, fns in tabs.items():
            if name == "natural_log_exp_and_others":
                out[name] = fns
            else:
                out[name] = {f for f in fns if f not in (Act.Exp, Act.Ln)}
        return out

    bacc.get_activation_tables = _tables_combined_exp_ln
    try:
        return _build_program_inner(mybir, tile, bacc)
    finally:
        bacc.get_activation_tables = _orig_tables


def _build_program_inner(mybir, tile, bacc):
    f32 = mybir.dt.float32
    f16 = mybir.dt.float16
    Alu = mybir.AluOpType
    Act = mybir.ActivationFunctionType

    nc = bacc.Bacc("TRN2", target_bir_lowering=False, debug=False)
    with tile.TileContext(nc) as tc:
        with tc.tile_pool(name="dram", bufs=1, space="DRAM") as dram:
            strm = dram.tile([BL, T, WROW], f16, kind="ExternalInput", name="strm")
            vpre_d = dram.tile([BL, T, VROW], f32, kind="ExternalInput", name="vpre")
            ea_d = dram.tile([BL, 4 * T], f32, kind="ExternalInput", name="ea")
            swrw_d = dram.tile([BL, 4 * T + 36], f32, kind="ExternalInput", name="swrw")
            initx = dram.tile([BL, 6], f32, kind="ExternalInput", name="initx")
            minit_d = dram.tile([BL, 2 * C], f16, kind="ExternalInput", name="minit")
            out_d = dram.tile([BL, 2 * T], f32, kind="ExternalOutput", name="out")

            with (
                tc.tile_pool(name="persist", bufs=1) as pp,
                tc.tile_pool(name="gat", bufs=3) as gp,
                tc.tile_pool(name="ring", bufs=4) as rp,
                tc.tile_pool(name="epi", bufs=1) as ep_,
            ):
                mA = pp.tile([BL, 2 * C], f16, name="mA")
                mB = pp.tile([BL, 2 * C], f16, name="mB")
                a3buf = pp.tile([BL, 2 * (T + 2)], f32, name="a3buf")
                qbuf = pp.tile([BL, 2 * T], f32, name="qbuf")
                ea_sb = pp.tile([BL, 4 * T], f32, name="ea_sb")
                swrw = pp.tile([BL, 4 * T], f32, name="swrw_sb")

                def sw2_col(t):
                    return swrw[:, t : t + 1]

                def swd_col(t):
                    return swrw[:, T + t : T + t + 1]

                def rw_col(t):
                    return swrw[:, 2 * T + t : 2 * T + t + 1]

                def get_gt(ch):
                    v = gp.tile([BL, CH, VROW], f32, name=f"vt{ch}", tag="vt")
                    nc.sync.dma_start(v[:], vpre_d[:, ch * CH : (ch + 1) * CH, :])
                    g = gp.tile([BL, CH, WROW], f16, name=f"gt{ch}", tag="gt")
                    nc.sync.dma_start(g[:], strm[:, ch * CH : (ch + 1) * CH, :])
                    return g, v

                m_cur, m_nxt = mA, mB
                prev_gt = None
                # startup: tiny/urgent loads first so the chain and its gap
                # ops can start within ~2us; the bulk chunk-0 stream follows.
                FE = 12
                v0 = gp.tile([BL, CH, VROW], f32, name="vt0", tag="vt")
                nc.sync.dma_start(v0[:], vpre_d[:, 0:CH, :])
                # mini swrw load: the FE leading cols of Sw2/SwD/Rw in ONE
                # strided DMA (host packs them contiguously at col 4T)
                sw3 = swrw[:, 0 : 3 * T].rearrange("p (sec t) -> p sec t", t=T)
                sw3_d = swrw_d[:, 0 : 3 * T].rearrange("p (sec t) -> p sec t", t=T)
                nc.sync.dma_start(
                    sw3[:, :, 0:FE],
                    swrw_d[:, 4 * T : 4 * T + 3 * FE].rearrange(
                        "p (sec t) -> p sec t", t=FE
                    ),
                )
                ge = pp.tile([BL, FE, WROW], f16, name="gearly")
                nc.sync.dma_start(ge[:], strm[:, 0:FE, :])
                nc.sync.dma_start(mA[:], minit_d[:])
                nc.sync.dma_start(qbuf[:, 0:2], initx[:, 0:2])
                nc.sync.dma_start(a3buf[:, 0:4], initx[:, 2:6])
                g0 = gp.tile([BL, CH, WROW], f16, name="gt0", tag="gt")
                nc.sync.dma_start(g0[:], strm[:, 0:CH, :])
                nc.sync.dma_start(sw3[:, :, FE:T], sw3_d[:, :, FE:T])
                nc.sync.dma_start(swrw[:, 3 * T : 4 * T], swrw_d[:, 3 * T : 4 * T])
                cur_gt, cur_vt = g0, v0
                SPd_prev = None
                vq2lo_prev = None
                dvz_prev = None

                for ch in range(NCH):
                    nxt_gt, nxt_vt = get_gt(ch + 1) if ch + 1 < NCH else (None, None)
                    if ch == 2:
                        nc.sync.dma_start(ea_sb[:], ea_d[:])
                    for j in range(CH):
                        t = ch * CH + j
                        if t > T - 2:
                            break
                        w_ap = ge[:, t, 0:64] if t < FE else cur_gt[:, j, 0:64]
                        if 1 <= t <= FE:
                            omc_prev_ap = ge[:, t - 1, 64:128]
                        elif j >= 1:
                            omc_prev_ap = cur_gt[:, j - 1, 64:128]
                        else:
                            omc_prev_ap = (
                                prev_gt[:, CH - 1, 64:128]
                                if prev_gt is not None
                                else None
                            )
                        vpre_lo_next = (
                            cur_vt[:, j + 1, 0:2] if j + 1 < CH else nxt_vt[:, 0, 0:2]
                        )
                        dvpre_next = (
                            cur_vt[:, j + 1, 4:6] if j + 1 < CH else nxt_vt[:, 0, 4:6]
                        )

                        # ---- chain: u = (SPd1-SPd0)*Rw (DVE), then ACT pair ----
                        ed = rp.tile([BL, 2], f32, name="ed", tag="ed")
                        SPd = rp.tile([BL, 2], f32, name="SPd", tag="SPd")
                        if t == 0:
                            # d_0 = dvpre_0 (a2_0 folded on host); bias 0
                            nc.scalar.activation(ed[:], cur_vt[:, 0, 4:6], Act.Exp)
                            ev_lo = cur_vt[:, 0, 0:2]
                        else:
                            ut = rp.tile([BL, 1], f32, name="ut", tag="ut")
                            nc.vector.scalar_tensor_tensor(
                                out=ut[:],
                                in0=SPd_prev[:, 1:2],
                                scalar=SPd_prev[:, 0:1],
                                in1=rw_col(t - 1),
                                op0=Alu.subtract,
                                op1=Alu.mult,
                            )
                            nc.scalar.activation(
                                ed[:], dvz_prev[:], Act.Exp, bias=ut[:, 0:1]
                            )
                            # ev_lo_t = SPd0_{t-1}*Rw_{t-1} + vq2lo_t (Pool pair)
                            e1 = rp.tile([BL, 1], f32, name="e1", tag="e1")
                            nc.gpsimd.tensor_tensor(
                                out=e1[:],
                                in0=SPd_prev[:, 0:1],
                                in1=rw_col(t - 1),
                                op=Alu.mult,
                            )
                            evl = rp.tile([BL, 2], f32, name="evl", tag="evl")
                            nc.gpsimd.tensor_tensor(
                                out=evl[:],
                                in0=vq2lo_prev[:],
                                in1=e1[:, 0:1].to_broadcast([BL, 2]),
                                op=Alu.add,
                            )
                            ev_lo = evl[:]
                        nc.scalar.activation(SPd[:], ed[:], Act.Ln, bias=1.0)

                        # ---- gap ops ----
                        # delta_{t-1} (Pool, ready immediately)
                        if t >= 1:
                            dl = rp.tile([BL, 2], f32, name="dl", tag="dl")
                            nc.gpsimd.tensor_tensor(
                                out=dl[:],
                                in0=a3buf[:, 2 * t + 2 : 2 * t + 4],
                                in1=a3buf[:, 2 * t : 2 * t + 2],
                                op=Alu.subtract,
                            )
                        # Wm2_t (DVE, accum) over m_{t-2} (cur, pre-update)
                        wm = rp.tile([BL, 2], f32, name="wm", tag="wm")
                        for s in range(2):
                            junk = rp.tile([BL, C], f16, name="junk", tag="junk")
                            nc.vector.scalar_tensor_tensor(
                                out=junk[:],
                                in0=m_cur[:, s * C : (s + 1) * C],
                                scalar=0.0,
                                in1=w_ap,
                                op0=Alu.add,
                                op1=Alu.mult,
                                accum_out=wm[:, s : s + 1],
                            )
                        # q1 = a3_{t-2}*Sw2_t + Wm2_t (DVE)
                        q1 = rp.tile([BL, 2], f32, name="q1", tag="q1")
                        nc.vector.scalar_tensor_tensor(
                            out=q1[:],
                            in0=a3buf[:, 2 * t : 2 * t + 2],
                            scalar=sw2_col(t),
                            in1=wm[:],
                            op0=Alu.mult,
                            op1=Alu.add,
                        )
                        # q_{t+1} = a3_{t-1}*SwD_t + q1 (DVE)
                        nc.vector.scalar_tensor_tensor(
                            out=qbuf[:, 2 * (t + 1) : 2 * (t + 1) + 2],
                            in0=a3buf[:, 2 * t + 2 : 2 * t + 4],
                            scalar=swd_col(t),
                            in1=q1[:],
                            op0=Alu.mult,
                            op1=Alu.add,
                        )
                        if t <= T - 3:
                            # z_{t+1} = ev_lo*Rw_t + q_{t+1} (Pool pair)
                            zp = rp.tile([BL, 2], f32, name="zp", tag="zp")
                            nc.gpsimd.tensor_tensor(
                                out=zp[:],
                                in0=ev_lo,
                                in1=rw_col(t).to_broadcast([BL, 2]),
                                op=Alu.mult,
                            )
                            zt = rp.tile([BL, 2], f32, name="zt", tag="zt")
                            nc.gpsimd.tensor_tensor(
                                out=zt[:],
                                in0=zp[:],
                                in1=qbuf[:, 2 * (t + 1) : 2 * (t + 1) + 2],
                                op=Alu.add,
                            )
                            # dz = z1 - z0 (Pool)
                            dz = rp.tile([BL, 1], f32, name="dz", tag="dz")
                            nc.gpsimd.tensor_tensor(
                                out=dz[:], in0=zt[:, 1:2], in1=zt[:, 0:1],
                                op=Alu.subtract,
                            )
                            # vq2lo_{t+1} = vpre_lo_{t+1} + z0 (Pool)
                            vq2lo = rp.tile([BL, 2], f32, name="vq2lo", tag="vq2lo")
                            nc.gpsimd.tensor_tensor(
                                out=vq2lo[:],
                                in0=vpre_lo_next,
                                in1=zt[:, 0:1].to_broadcast([BL, 2]),
                                op=Alu.add,
                            )
                            # dvz_{t+1} = dvpre_{t+1} + dz (Pool)
                            dvz = rp.tile([BL, 2], f32, name="dvz", tag="dvz")
                            nc.gpsimd.tensor_tensor(
                                out=dvz[:],
                                in0=dvpre_next,
                                in1=dz[:, 0:1].to_broadcast([BL, 2]),
                                op=Alu.add,
                            )
                            vq2lo_prev = vq2lo
                            dvz_prev = dvz
                        # m-upd_{t-1} (DVE, end of step): m_{t-1}=(m_{t-2}-d)*omc_{t-1}
                        if t >= 1 and t <= T - 3:
                            for s in range(2):
                                nc.vector.scalar_tensor_tensor(
                                    out=m_nxt[:, s * C : (s + 1) * C],
                                    in0=m_cur[:, s * C : (s + 1) * C],
                                    scalar=dl[:, s : s + 1],
                                    in1=omc_prev_ap,
                                    op0=Alu.subtract,
                                    op1=Alu.mult,
                                )
                            m_cur, m_nxt = m_nxt, m_cur
                        # a3add_t (Pool): a3buf col-pair t+2 = a3_t
                        nc.gpsimd.tensor_tensor(
                            out=a3buf[:, 2 * t + 4 : 2 * t + 6],
                            in0=ev_lo,
                            in1=SPd[:],
                            op=Alu.add,
                        )
                        SPd_prev = SPd
                    prev_gt = cur_gt
                    cur_gt, cur_vt = nxt_gt, nxt_vt

                # ---- epilogue: outputs for all t (pipelined halves) ----
                for h, (t0, TH) in enumerate(
                    [(0, 140), (140, 140), (280, 140), (420, 80)]
                ):
                    tmp = ep_.tile([BL, 2 * TH], f32, name=f"tmp{h}")
                    nc.gpsimd.tensor_tensor(
                        out=tmp[:].rearrange("p (t j) -> p t j", j=2),
                        in0=a3buf[:, 2 * t0 + 2 : 2 * (t0 + TH) + 2].rearrange(
                            "p (t j) -> p t j", j=2
                        ),
                        in1=swrw[:, 3 * T + t0 : 3 * T + t0 + TH]
                        .rearrange("p (t one) -> p t one", one=1)
                        .to_broadcast([BL, TH, 2]),
                        op=Alu.mult,
                    )
                    a2all = ep_.tile([BL, 2 * TH], f32, name=f"a2all{h}")
                    nc.gpsimd.tensor_tensor(
                        out=a2all[:],
                        in0=tmp[:],
                        in1=qbuf[:, 2 * t0 : 2 * (t0 + TH)],
                        op=Alu.add,
                    )
                    earg = ep_.tile([BL, 4 * TH], f32, name=f"earg{h}")
                    nc.gpsimd.tensor_tensor(
                        out=earg[:].rearrange("p (t s o) -> p t s o", s=2, o=2),
                        in0=ea_sb[:, 4 * t0 : 4 * (t0 + TH)].rearrange(
                            "p (t s o) -> p t s o", s=2, o=2
                        ),
                        in1=a2all[:]
                        .rearrange("p (t s one) -> p t s one", s=2, one=1)
                        .to_broadcast([BL, TH, 2, 2]),
                        op=Alu.add,
                    )
                    epv = ep_.tile([BL, 4 * TH], f32, name=f"epv{h}")
                    nc.scalar.activation(epv[:], earg[:], Act.Exp)
                    epvR = epv[:].rearrange("p (t s o) -> p t s o", s=2, o=2)
                    smb = ep_.tile([BL, 3 * TH], f32, name=f"smb{h}")
                    smbR = smb[:].rearrange("p (t k) -> p t k", k=3)
                    nc.gpsimd.tensor_tensor(
                        out=smbR[:, :, 0:2],
                        in0=epvR[:, :, 0, :],
                        in1=epvR[:, :, 1, :],
                        op=Alu.add,
                    )
                    nc.gpsimd.tensor_add(smbR[:, :, 2], smbR[:, :, 0], smbR[:, :, 1])
                    lgb = ep_.tile([BL, 3 * TH], f32, name=f"lgb{h}")
                    nc.scalar.activation(lgb[:], smb[:], Act.Ln)
                    lgbR = lgb[:].rearrange("p (t k) -> p t k", k=3)
                    ob = ep_.tile([BL, 2 * TH], f32, name=f"ob{h}")
                    nc.gpsimd.tensor_tensor(
                        out=ob[:].rearrange("p (t o) -> p t o", o=2),
                        in0=lgbR[:, :, 0:2],
                        in1=lgbR[:, :, 2:3].to_broadcast([BL, TH, 2]),
                        op=Alu.subtract,
                    )
                    nc.sync.dma_start(
                        out_d[:, 2 * t0 : 2 * (t0 + TH)], ob[:]
                    )
    nc.compile()
    names = dict(
        strm=strm.tensor.name,
        vpre=vpre_d.tensor.name,
        ea=ea_d.tensor.name,
        swrw=swrw_d.tensor.name,
        initx=initx.tensor.name,
        minit=minit_d.tensor.name,
        out=out_d.tensor.name,
    )
    return nc, names


def _get_program():
    with _lock:
        if "nc" not in _cache:
            _cache["nc"], _cache["names"] = _build_program()
    return _cache["nc"], _cache["names"]


def _log_softmax(x, axis):
    x = x.astype(np.float64)
    m = x.max(axis=axis, keepdims=True)
    e = np.exp(x - m)
    return x - m - np.log(e.sum(axis=axis, keepdims=True))


def _host_prep(corr, kc, A, trans_logits, obs_logits, init_logits):
    A = np.asarray(A, np.float64)                       # [K,C]
    kc = np.asarray(kc, np.int64)
    corr = np.asarray(corr, np.int64)
    log_obs = _log_softmax(np.asarray(obs_logits), 2)   # [C,S,O]
    log_t = _log_softmax(np.asarray(trans_logits), 1)   # [C,S,S]
    log_i = _log_softmax(np.asarray(init_logits), 1)    # [C,S]
    AW = A @ log_obs.reshape(C, 4)                      # [K,4] cols s*2+o
    AT = A @ log_t.reshape(C, 4)                        # [K,4] cols s*2+t'
    AI = A @ log_i                                      # [K,2]

    f = np.float32
    h = np.float16
    Acc = A[kc].astype(f)                               # [B,T,64]
    w = np.zeros((B, T, 64), f)
    w[:, :-1] = Acc[:, 1:] * (1.0 - Acc[:, :-1])
    omc = (1.0 - Acc).astype(f)
    w2 = w.copy()
    w2[:, 1:] *= omc[:, :-1]
    strm = np.zeros((B, T, WROW), h)
    strm[:, :, 0:64] = w2.astype(h)                     # w2 = w*omc_{t-1}
    strm[:, :, 64:128] = omc.astype(h)
    ATg = AT[kc].astype(f)
    AWg = AW[kc].astype(f)
    ao_y0 = np.take_along_axis(AWg, (0 + corr)[..., None], axis=2)[..., 0]
    ao_y1 = np.take_along_axis(AWg, (2 + corr)[..., None], axis=2)[..., 0]
    vpre = np.zeros((B, T, VROW), f)
    vpre[:, :, 0] = ATg[..., 0] + ao_y0                 # (t'=0,s=0)
    vpre[:, :, 1] = ATg[..., 2] + ao_y0                 # (t'=0,s=1)
    vpre[:, :, 2] = ATg[..., 1] + ao_y1                 # (t'=1,s=0)
    vpre[:, :, 3] = ATg[..., 3] + ao_y1                 # (t'=1,s=1)
    a2_0 = AI[kc[:, 0]].astype(f)                       # [B,2]
    vpre[:, 0, 0] += a2_0[:, 0]
    vpre[:, 0, 1] += a2_0[:, 0]
    vpre[:, 0, 2] += a2_0[:, 1]
    vpre[:, 0, 3] += a2_0[:, 1]
    vpre[:, :, 4] = vpre[:, :, 2] - vpre[:, :, 0]       # dvpre (post a2_0 fold)
    vpre[:, :, 5] = vpre[:, :, 3] - vpre[:, :, 1]

    ea = AWg.reshape(B, 4 * T)                          # [B,4T]
    Sw = w.sum(-1)                                      # [B,T]
    Sw2 = w2.astype(h).astype(f).sum(-1)                # match device fp16 w2
    Rw = 1.0 - Sw
    swrw = np.zeros((B, 4 * T + 36), f)
    swrw[:, 0:T] = Sw2
    swrw[:, T : 2 * T] = Sw - Sw2                       # SwD
    swrw[:, 2 * T : 3 * T] = Rw
    swrw[:, 3 * T + 1 : 4 * T] = Rw[:, :-1]             # RwShift (col0 = 0)
    for sec in range(3):                                # packed FE mini-cols
        swrw[:, 4 * T + 12 * sec : 4 * T + 12 * (sec + 1)] = \
            swrw[:, sec * T : sec * T + 12]

    initx = np.zeros((B, 6), f)
    initx[:, 0:2] = a2_0
    # cols 2:6 stay zero -> a3buf zero col-pairs (a3_{-2}, a3_{-1})
    minit = np.zeros((B, 2 * C), h)
    minit[:, 0:64] = log_i[:, 0].astype(h)[None, :]
    minit[:, 64:128] = log_i[:, 1].astype(h)[None, :]
    return strm, vpre, ea, swrw, initx, minit


def build_in_maps(inputs):
    nc, names = _get_program()
    strm, vpre, ea, swrw, initx, minit = _host_prep(**inputs)
    in_maps = []
    for c in range(N_CORES):
        sl = slice(c * BL, (c + 1) * BL)
        in_maps.append(
            {
                names["strm"]: strm[sl],
                names["vpre"]: vpre[sl],
                names["ea"]: ea[sl],
                names["swrw"]: swrw[sl],
                names["initx"]: initx[sl],
                names["minit"]: minit[sl],
            }
        )
    return nc, names, in_maps


def kernel(corr, kc, A, trans_logits, obs_logits, init_logits):
    from concourse.bass_utils import run_bass_kernel_spmd

    nc, names, in_maps = build_in_maps(
        dict(corr=corr, kc=kc, A=A, trans_logits=trans_logits,
             obs_logits=obs_logits, init_logits=init_logits)
    )
    res = run_bass_kernel_spmd(nc, in_maps, core_ids=list(range(N_CORES)))
    outs = [res.results[c][names["out"]].reshape(BL, T, O) for c in range(N_CORES)]
    return np.concatenate(outs, axis=0)


# revision 42
# speedup vs baseline: 1.0010x; 1.0010x over previous
"""Trainium2 Bass kernel for nn_BktModel (soft-membership BKT HMM forward), v2.

Math restructure (exact; validated in mathcheck.py):
  State m_t[c,s] = la_t[c,s] - a3_t[s]  (shifted log-alpha; a3_{-1}=0, m_{-1}=log_i).
  With w_t = cc_{t+1}*omc_t, Sw_t = sum_c w_t, Rw_t = 1-Sw_t (host-precomputed):
    a2_{t+1}[j] = Wm_t[j] + a3_{t-1}[j]*Sw_t + a3_t[j]*Rw_t,  Wm_t = sum_c w_t*m_{t-1}
    m_t = (m_{t-1} - delta_t) * omc_t,  delta_t = a3_t - a3_{t-1}
  Per-step scalar chain (S=2 collapses logsumexp to softplus):
    evarg_t[t',s] = vpre_t[t',s] + a2_t[t']
    a3_t[s] = evarg_t[0,s] + ln(1 + exp(d_t[s])),  d_t = evarg_t[1,:] - evarg_t[0,:]
  Softplus = ACT Exp then ACT Ln(bias=1) back-to-back in one act-table set, so the
  critical chain per step is [DVE evarg-stt, DVE d] -> [ACT exp, ACT ln1p].
  The lazy split q_{t+1} = Wm_t + a3_{t-1}*Sw_t, z_{t+1} = q_{t+1} + evarg0_t*Rw_t,
  vq2_{t+1} = vpre_{t+1} + z_{t+1} keeps everything else off the chain.
  v3: Wm_t is further decoupled from the fresh state via
    Wm_t = Wm2_t - delta_{t-1}*Sw2_t,  Wm2_t = sum_c w2_t*m_{t-2}, w2_t = w_t*omc_{t-1}
  (w2/Sw2/SwD=Sw-Sw2 host-precomputed) so the per-step DVE cascade
  Wm2 -> q1 -> q -> z -> vq2 starts immediately instead of waiting on
  ln -> delta -> m-update; the m-update drops to the end of the step.
  All output work (a2 = q + a3*RwShift, exp, log-softmax) runs as one batched
  epilogue after the T-loop; a2 inputs (qbuf, a3buf) are written per step.

Sharding: data-parallel over batch, 8 cores x 128 rows (partition dim = batch).
"""

import os
import sys
import threading

import numpy as np

for _p in ("/opt/trn_rl_repo", "/root/.axon_site/_ro/trn_rl_repo"):
    if os.path.isdir(_p) and _p not in sys.path:
        sys.path.append(_p)

B, T, C, K = 1024, 500, 64, 2000
S, O = 2, 2
N_CORES = 8
BL = B // N_CORES          # local batch per core (= 128 partitions)
CH = 25                    # timesteps per streamed chunk
NCH = T // CH
WROW = 128                 # fp16 halves per streamed row: w2(64) omc(64)
VROW = 6                   # fp32 floats per streamed row: vpre(4) dvpre(2)

_cache = {}
_lock = threading.Lock()


def _build_program():
    import concourse.mybir as mybir
    import concourse.tile as tile
    from concourse import bacc

    Act = mybir.ActivationFunctionType

    # Steer Bacc's act-table pass to the set holding BOTH Exp and Ln (the only
    # transcendentals used); avoids any activation-table reloads.
    _orig_tables = bacc.get_activation_tables

    def _tables_combined_exp_ln(arch):
        tabs = _orig_tables(arch)
        out = {}
        for name, fns in tabs.items():
            if name == "natural_log_exp_and_others":
                out[name] = fns
            else:
                out[name] = {f for f in fns if f not in (Act.Exp, Act.Ln)}
        return out

    bacc.get_activation_tables = _tables_combined_exp_ln
    try:
        return _build_program_inner(mybir, tile, bacc)
    finally:
        bacc.get_activation_tables = _orig_tables


def _build_program_inner(mybir, tile, bacc):
    f32 = mybir.dt.float32
    f16 = mybir.dt.float16
    Alu = mybir.AluOpType
    Act = mybir.ActivationFunctionType

    nc = bacc.Bacc("TRN2", target_bir_lowering=False, debug=False)
    with tile.TileContext(nc) as tc:
        with tc.tile_pool(name="dram", bufs=1, space="DRAM") as dram:
            strm = dram.tile([BL, T, WROW], f16, kind="ExternalInput", name="strm")
            vpre_d = dram.tile([BL, T, VROW], f32, kind="ExternalInput", name="vpre")
            ea_d = dram.tile([BL, 4 * T], f32, kind="ExternalInput", name="ea")
            swrw_d = dram.tile([BL, 4 * T + 36], f32, kind="ExternalInput", name="swrw")
            initx = dram.tile([BL, 6], f32, kind="ExternalInput", name="initx")
            minit_d = dram.tile([BL, 2 * C], f16, kind="ExternalInput", name="minit")
            out_d = dram.tile([BL, 2 * T], f32, kind="ExternalOutput", name="out")

            with (
                tc.tile_pool(name="persist", bufs=1) as pp,
                tc.tile_pool(name="gat", bufs=3) as gp,
                tc.tile_pool(name="ring", bufs=4) as rp,
                tc.tile_pool(name="epi", bufs=1) as ep_,
            ):
                mA = pp.tile([BL, 2 * C], f16, name="mA")
                mB = pp.tile([BL, 2 * C], f16, name="mB")
                a3buf = pp.tile([BL, 2 * (T + 2)], f32, name="a3buf")
                qbuf = pp.tile([BL, 2 * T], f32, name="qbuf")
                ea_sb = pp.tile([BL, 4 * T], f32, name="ea_sb")
                swrw = pp.tile([BL, 4 * T], f32, name="swrw_sb")

                def sw2_col(t):
                    return swrw[:, t : t + 1]

                def swd_col(t):
                    return swrw[:, T + t : T + t + 1]

                def rw_col(t):
                    return swrw[:, 2 * T + t : 2 * T + t + 1]

                def get_gt(ch):
                    v = gp.tile([BL, CH, VROW], f32, name=f"vt{ch}", tag="vt")
                    nc.sync.dma_start(v[:], vpre_d[:, ch * CH : (ch + 1) * CH, :])
                    g = gp.tile([BL, CH, WROW], f16, name=f"gt{ch}", tag="gt")
                    nc.sync.dma_start(g[:], strm[:, ch * CH : (ch + 1) * CH, :])
                    return g, v

                m_cur, m_nxt = mA, mB
                prev_gt = None
                # startup: tiny/urgent loads first so the chain and its gap
                # ops can start within ~2us; the bulk chunk-0 stream follows.
                FE = 12
                v0 = gp.tile([BL, CH, VROW], f32, name="vt0", tag="vt")
                nc.sync.dma_start(v0[:], vpre_d[:, 0:CH, :])
                # mini swrw load: the FE leading cols of Sw2/SwD/Rw in ONE
                # strided DMA (host packs them contiguously at col 4T)
                sw3 = swrw[:, 0 : 3 * T].rearrange("p (sec t) -> p sec t", t=T)
                sw3_d = swrw_d[:, 0 : 3 * T].rearrange("p (sec t) -> p sec t", t=T)
                nc.sync.dma_start(
                    sw3[:, :, 0:FE],
                    swrw_d[:, 4 * T : 4 * T + 3 * FE].rearrange(
                        "p (sec t) -> p sec t", t=FE
                    ),
                )
                ge = pp.tile([BL, FE, WROW], f16, name="gearly")
                nc.sync.dma_start(ge[:], strm[:, 0:FE, :])
                nc.sync.dma_start(mA[:], minit_d[:])
                nc.sync.dma_start(qbuf[:, 0:2], initx[:, 0:2])
                nc.sync.dma_start(a3buf[:, 0:4], initx[:, 2:6])
                g0 = gp.tile([BL, CH, WROW], f16, name="gt0", tag="gt")
                nc.sync.dma_start(g0[:], strm[:, 0:CH, :])
                nc.sync.dma_start(sw3[:, :, FE:T], sw3_d[:, :, FE:T])
                nc.sync.dma_start(swrw[:, 3 * T : 4 * T], swrw_d[:, 3 * T : 4 * T])
                cur_gt, cur_vt = g0, v0
                SPd_prev = None
                vq2lo_prev = None
                dvz_prev = None

                for ch in range(NCH):
                    nxt_gt, nxt_vt = get_gt(ch + 1) if ch + 1 < NCH else (None, None)
                    if ch == 2:
                        nc.sync.dma_start(ea_sb[:], ea_d[:])
                    for j in range(CH):
                        t = ch * CH + j
                        if t > T - 2:
                            break
                        w_ap = ge[:, t, 0:64] if t < FE else cur_gt[:, j, 0:64]
                        if 1 <= t <= FE:
                            omc_prev_ap = ge[:, t - 1, 64:128]
                        elif j >= 1:
                            omc_prev_ap = cur_gt[:, j - 1, 64:128]
                        else:
                            omc_prev_ap = (
                                prev_gt[:, CH - 1, 64:128]
                                if prev_gt is not None
                                else None
                            )
                        vpre_lo_next = (
                            cur_vt[:, j + 1, 0:2] if j + 1 < CH else nxt_vt[:, 0, 0:2]
                        )
                        dvpre_next = (
                            cur_vt[:, j + 1, 4:6] if j + 1 < CH else nxt_vt[:, 0, 4:6]
                        )

                        # ---- chain: u = (SPd1-SPd0)*Rw (DVE), then ACT pair ----
                        ed = rp.tile([BL, 2], f32, name="ed", tag="ed")
                        SPd = rp.tile([BL, 2], f32, name="SPd", tag="SPd")
                        if t == 0:
                            # d_0 = dvpre_0 (a2_0 folded on host); bias 0
                            nc.scalar.activation(ed[:], cur_vt[:, 0, 4:6], Act.Exp)
                            ev_lo = cur_vt[:, 0, 0:2]
                        else:
                            ut = rp.tile([BL, 1], f32, name="ut", tag="ut")
                            nc.vector.scalar_tensor_tensor(
                                out=ut[:],
                                in0=SPd_prev[:, 1:2],
                                scalar=SPd_prev[:, 0:1],
                                in1=rw_col(t - 1),
                                op0=Alu.subtract,
                                op1=Alu.mult,
                            )
                            nc.scalar.activation(
                                ed[:], dvz_prev[:], Act.Exp, bias=ut[:, 0:1]
                            )
                            # ev_lo_t = SPd0_{t-1}*Rw_{t-1} + vq2lo_t (Pool pair)
                            e1 = rp.tile([BL, 1], f32, name="e1", tag="e1")
                            nc.gpsimd.tensor_tensor(
                                out=e1[:],
                                in0=SPd_prev[:, 0:1],
                                in1=rw_col(t - 1),
                                op=Alu.mult,
                            )
                            evl = rp.tile([BL, 2], f32, name="evl", tag="evl")
                            nc.gpsimd.tensor_tensor(
                                out=evl[:],
                                in0=vq2lo_prev[:],
                                in1=e1[:, 0:1].to_broadcast([BL, 2]),
                                op=Alu.add,
                            )
                            ev_lo = evl[:]
                        nc.scalar.activation(SPd[:], ed[:], Act.Ln, bias=1.0)

                        # ---- gap ops ----
                        # delta_{t-1} (Pool, ready immediately)
                        if t >= 1:
                            dl = rp.tile([BL, 2], f32, name="dl", tag="dl")
                            nc.gpsimd.tensor_tensor(
                                out=dl[:],
                                in0=a3buf[:, 2 * t + 2 : 2 * t + 4],
                                in1=a3buf[:, 2 * t : 2 * t + 2],
                                op=Alu.subtract,
                            )
                        # Wm2_t (DVE, accum) over m_{t-2} (cur, pre-update)
                        wm = rp.tile([BL, 2], f32, name="wm", tag="wm")
                        for s in range(2):
                            junk = rp.tile([BL, C], f16, name="junk", tag="junk")
                            nc.vector.scalar_tensor_tensor(
                                out=junk[:],
                                in0=m_cur[:, s * C : (s + 1) * C],
                                scalar=0.0,
                                in1=w_ap,
                                op0=Alu.add,
                                op1=Alu.mult,
                                accum_out=wm[:, s : s + 1],
                            )
                        # q1 = a3_{t-2}*Sw2_t + Wm2_t (DVE)
                        q1 = rp.tile([BL, 2], f32, name="q1", tag="q1")
                        nc.vector.scalar_tensor_tensor(
                            out=q1[:],
                            in0=a3buf[:, 2 * t : 2 * t + 2],
                            scalar=sw2_col(t),
                            in1=wm[:],
                            op0=Alu.mult,
                            op1=Alu.add,
                        )
                        # q_{t+1} = a3_{t-1}*SwD_t + q1 (DVE)
                        nc.vector.scalar_tensor_tensor(
                            out=qbuf[:, 2 * (t + 1) : 2 * (t + 1) + 2],
                            in0=a3buf[:, 2 * t + 2 : 2 * t + 4],
                            scalar=swd_col(t),
                            in1=q1[:],
                            op0=Alu.mult,
                            op1=Alu.add,
                        )
                        if t <= T - 3:
                            # z_{t+1} = ev_lo*Rw_t + q_{t+1} (Pool pair)
                            zp = rp.tile([BL, 2], f32, name="zp", tag="zp")
                            nc.gpsimd.tensor_tensor(
                                out=zp[:],
                                in0=ev_lo,
                                in1=rw_col(t).to_broadcast([BL, 2]),
                                op=Alu.mult,
                            )
                            zt = rp.tile([BL, 2], f32, name="zt", tag="zt")
                            nc.gpsimd.tensor_tensor(
                                out=zt[:],
                                in0=zp[:],
                                in1=qbuf[:, 2 * (t + 1) : 2 * (t + 1) + 2],
                                op=Alu.add,
                            )
                            # dz = z1 - z0 (Pool)
                            dz = rp.tile([BL, 1], f32, name="dz", tag="dz")
                            nc.gpsimd.tensor_tensor(
                                out=dz[:], in0=zt[:, 1:2], in1=zt[:, 0:1],
                                op=Alu.subtract,
                            )
                            # vq2lo_{t+1} = vpre_lo_{t+1} + z0 (Pool)
                            vq2lo = rp.tile([BL, 2], f32, name="vq2lo", tag="vq2lo")
                            nc.gpsimd.tensor_tensor(
                                out=vq2lo[:],
                                in0=vpre_lo_next,
                                in1=zt[:, 0:1].to_broadcast([BL, 2]),
                                op=Alu.add,
                            )
                            # dvz_{t+1} = dvpre_{t+1} + dz (Pool)
                            dvz = rp.tile([BL, 2], f32, name="dvz", tag="dvz")
                            nc.gpsimd.tensor_tensor(
                                out=dvz[:],
                                in0=dvpre_next,
                                in1=dz[:, 0:1].to_broadcast([BL, 2]),
                                op=Alu.add,
                            )
                            vq2lo_prev = vq2lo
                            dvz_prev = dvz
                        # m-upd_{t-1} (DVE, end of step): m_{t-1}=(m_{t-2}-d)*omc_{t-1}
                        if t >= 1 and t <= T - 3:
                            for s in range(2):
                                nc.vector.scalar_tensor_tensor(
                                    out=m_nxt[:, s * C : (s + 1) * C],
                                    in0=m_cur[:, s * C : (s + 1) * C],
                                    scalar=dl[:, s : s + 1],
                                    in1=omc_prev_ap,
                                    op0=Alu.subtract,
                                    op1=Alu.mult,
                                )
                            m_cur, m_nxt = m_nxt, m_cur
                        # a3add_t (Pool): a3buf col-pair t+2 = a3_t
                        nc.gpsimd.tensor_tensor(
                            out=a3buf[:, 2 * t + 4 : 2 * t + 6],
                            in0=ev_lo,
                            in1=SPd[:],
                            op=Alu.add,
                        )
                        SPd_prev = SPd
                    prev_gt = cur_gt
                    cur_gt, cur_vt = nxt_gt, nxt_vt

                # ---- epilogue: outputs for all t (pipelined halves) ----
                TH = T // 4
                for h in range(4):
                    t0 = h * TH
                    tmp = ep_.tile([BL, 2 * TH], f32, name=f"tmp{h}")
                    nc.gpsimd.tensor_tensor(
                        out=tmp[:].rearrange("p (t j) -> p t j", j=2),
                        in0=a3buf[:, 2 * t0 + 2 : 2 * (t0 + TH) + 2].rearrange(
                            "p (t j) -> p t j", j=2
                        ),
                        in1=swrw[:, 3 * T + t0 : 3 * T + t0 + TH]
                        .rearrange("p (t one) -> p t one", one=1)
                        .to_broadcast([BL, TH, 2]),
                        op=Alu.mult,
                    )
                    a2all = ep_.tile([BL, 2 * TH], f32, name=f"a2all{h}")
                    nc.gpsimd.tensor_tensor(
                        out=a2all[:],
                        in0=tmp[:],
                        in1=qbuf[:, 2 * t0 : 2 * (t0 + TH)],
                        op=Alu.add,
                    )
                    earg = ep_.tile([BL, 4 * TH], f32, name=f"earg{h}")
                    nc.gpsimd.tensor_tensor(
                        out=earg[:].rearrange("p (t s o) -> p t s o", s=2, o=2),
                        in0=ea_sb[:, 4 * t0 : 4 * (t0 + TH)].rearrange(
                            "p (t s o) -> p t s o", s=2, o=2
                        ),
                        in1=a2all[:]
                        .rearrange("p (t s one) -> p t s one", s=2, one=1)
                        .to_broadcast([BL, TH, 2, 2]),
                        op=Alu.add,
                    )
                    epv = ep_.tile([BL, 4 * TH], f32, name=f"epv{h}")
                    nc.scalar.activation(epv[:], earg[:], Act.Exp)
                    epvR = epv[:].rearrange("p (t s o) -> p t s o", s=2, o=2)
                    smb = ep_.tile([BL, 3 * TH], f32, name=f"smb{h}")
                    smbR = smb[:].rearrange("p (t k) -> p t k", k=3)
                    nc.gpsimd.tensor_tensor(
                        out=smbR[:, :, 0:2],
                        in0=epvR[:, :, 0, :],
                        in1=epvR[:, :, 1, :],
                        op=Alu.add,
                    )
                    nc.gpsimd.tensor_add(smbR[:, :, 2], smbR[:, :, 0], smbR[:, :, 1])
                    lgb = ep_.tile([BL, 3 * TH], f32, name=f"lgb{h}")
                    nc.scalar.activation(lgb[:], smb[:], Act.Ln)
                    lgbR = lgb[:].rearrange("p (t k) -> p t k", k=3)
                    ob = ep_.tile([BL, 2 * TH], f32, name=f"ob{h}")
                    nc.gpsimd.tensor_tensor(
                        out=ob[:].rearrange("p (t o) -> p t o", o=2),
                        in0=lgbR[:, :, 0:2],
                        in1=lgbR[:, :, 2:3].to_broadcast([BL, TH, 2]),
                        op=Alu.subtract,
                    )
                    nc.sync.dma_start(
                        out_d[:, 2 * t0 : 2 * (t0 + TH)], ob[:]
                    )
    nc.compile()
    names = dict(
        strm=strm.tensor.name,
        vpre=vpre_d.tensor.name,
        ea=ea_d.tensor.name,
        swrw=swrw_d.tensor.name,
        initx=initx.tensor.name,
        minit=minit_d.tensor.name,
        out=out_d.tensor.name,
    )
    return nc, names


def _get_program():
    with _lock:
        if "nc" not in _cache:
            _cache["nc"], _cache["names"] = _build_program()
    return _cache["nc"], _cache["names"]


def _log_softmax(x, axis):
    x = x.astype(np.float64)
    m = x.max(axis=axis, keepdims=True)
    e = np.exp(x - m)
    return x - m - np.log(e.sum(axis=axis, keepdims=True))


def _host_prep(corr, kc, A, trans_logits, obs_logits, init_logits):
    A = np.asarray(A, np.float64)                       # [K,C]
    kc = np.asarray(kc, np.int64)
    corr = np.asarray(corr, np.int64)
    log_obs = _log_softmax(np.asarray(obs_logits), 2)   # [C,S,O]
    log_t = _log_softmax(np.asarray(trans_logits), 1)   # [C,S,S]
    log_i = _log_softmax(np.asarray(init_logits), 1)    # [C,S]
    AW = A @ log_obs.reshape(C, 4)                      # [K,4] cols s*2+o
    AT = A @ log_t.reshape(C, 4)                        # [K,4] cols s*2+t'
    AI = A @ log_i                                      # [K,2]

    f = np.float32
    h = np.float16
    Acc = A[kc].astype(f)                               # [B,T,64]
    w = np.zeros((B, T, 64), f)
    w[:, :-1] = Acc[:, 1:] * (1.0 - Acc[:, :-1])
    omc = (1.0 - Acc).astype(f)
    w2 = w.copy()
    w2[:, 1:] *= omc[:, :-1]
    strm = np.zeros((B, T, WROW), h)
    strm[:, :, 0:64] = w2.astype(h)                     # w2 = w*omc_{t-1}
    strm[:, :, 64:128] = omc.astype(h)
    ATg = AT[kc].astype(f)
    AWg = AW[kc].astype(f)
    ao_y0 = np.take_along_axis(AWg, (0 + corr)[..., None], axis=2)[..., 0]
    ao_y1 = np.take_along_axis(AWg, (2 + corr)[..., None], axis=2)[..., 0]
    vpre = np.zeros((B, T, VROW), f)
    vpre[:, :, 0] = ATg[..., 0] + ao_y0                 # (t'=0,s=0)
    vpre[:, :, 1] = ATg[..., 2] + ao_y0                 # (t'=0,s=1)
    vpre[:, :, 2] = ATg[..., 1] + ao_y1                 # (t'=1,s=0)
    vpre[:, :, 3] = ATg[..., 3] + ao_y1                 # (t'=1,s=1)
    a2_0 = AI[kc[:, 0]].astype(f)                       # [B,2]
    vpre[:, 0, 0] += a2_0[:, 0]
    vpre[:, 0, 1] += a2_0[:, 0]
    vpre[:, 0, 2] += a2_0[:, 1]
    vpre[:, 0, 3] += a2_0[:, 1]
    vpre[:, :, 4] = vpre[:, :, 2] - vpre[:, :, 0]       # dvpre (post a2_0 fold)
    vpre[:, :, 5] = vpre[:, :, 3] - vpre[:, :, 1]

    ea = AWg.reshape(B, 4 * T)                          # [B,4T]
    Sw = w.sum(-1)                                      # [B,T]
    Sw2 = w2.astype(h).astype(f).sum(-1)                # match device fp16 w2
    Rw = 1.0 - Sw
    swrw = np.zeros((B, 4 * T + 36), f)
    swrw[:, 0:T] = Sw2
    swrw[:, T : 2 * T] = Sw - Sw2                       # SwD
    swrw[:, 2 * T : 3 * T] = Rw
    swrw[:, 3 * T + 1 : 4 * T] = Rw[:, :-1]             # RwShift (col0 = 0)
    for sec in range(3):                                # packed FE mini-cols
        swrw[:, 4 * T + 12 * sec : 4 * T + 12 * (sec + 1)] = \
            swrw[:, sec * T : sec * T + 12]

    initx = np.zeros((B, 6), f)
    initx[:, 0:2] = a2_0
    # cols 2:6 stay zero -> a3buf zero col-pairs (a3_{-2}, a3_{-1})
    minit = np.zeros((B, 2 * C), h)
    minit[:, 0:64] = log_i[:, 0].astype(h)[None, :]
    minit[:, 64:128] = log_i[:, 1].astype(h)[None, :]
    return strm, vpre, ea, swrw, initx, minit


def build_in_maps(inputs):
    nc, names = _get_program()
    strm, vpre, ea, swrw, initx, minit = _host_prep(**inputs)
    in_maps = []
    for c in range(N_CORES):
        sl = slice(c * BL, (c + 1) * BL)
        in_maps.append(
            {
                names["strm"]: strm[sl],
                names["vpre"]: vpre[sl],
                names["ea"]: ea[sl],
                names["swrw"]: swrw[sl],
                names["initx"]: initx[sl],
                names["minit"]: minit[sl],
            }
        )
    return nc, names, in_maps


def kernel(corr, kc, A, trans_logits, obs_logits, init_logits):
    from concourse.bass_utils import run_bass_kernel_spmd

    nc, names, in_maps = build_in_maps(
        dict(corr=corr, kc=kc, A=A, trans_logits=trans_logits,
             obs_logits=obs_logits, init_logits=init_logits)
    )
    res = run_bass_kernel_spmd(nc, in_maps, core_ids=list(range(N_CORES)))
    outs = [res.results[c][names["out"]].reshape(BL, T, O) for c in range(N_CORES)]
    return np.concatenate(outs, axis=0)
